# revision 9
# baseline (speedup 1.0000x reference)
"""MultiHeadAttention forward on 8 Trainium2 NeuronCores (Bass/Tile).

Problem: B=2, S=2048, D=1024, H=16 heads (dk=64), fp32, mask all-ones.

Sharding: core c = b*4 + g handles batch b and head group g (4 heads).
Data parallel over B, tensor parallel over heads; w_o row-wise with the
partial-output reduction done host-side (summing 4 fp32 partials).

Device kernel per core, bf16 operands with fp32 PSUM accumulation.
The body is 8 attention passes (4 q-chunks x 2 head-pairs); ALL
projections are software-pipelined across iterations as PE filler ops
interleaved between the exp and PV steps of each pass, so the PE never
sits in a projection-only phase while the ACT engine is idle:
  - q-projection of chunk qc+1 runs inside chunk qc's pr=1 pass.
  - output projection of chunk qc-1 runs inside chunk qc's pr=0 pass.
  - k/v projections for the NEXT iteration run spread over all passes,
    writing a second khT/vh buffer generation (the loop body is
    unrolled an even number of times; generations alternate).
Filler matmul groups accumulate in 1-bank [128,512] PSUM tiles (pool
bufs=2) so group handoffs never stall on a shared accumulator.
Scores for the next pass are issued in the last slot of the current
pass so exp(0) of the next pass hides under the pass boundary.
"""
import math

import numpy as np

B, S, D, H = 2, 2048, 1024, 16
DK = D // H          # 64
HPC = H // 4         # 4 heads per core
NCORES = 8
NT = S // 128        # 16 k-tiles / s-tiles
ND = D // 128        # 8 d-tiles
QC = 512             # q-chunk
NQC = S // QC        # 4
GH = HPC * DK        # 256 output dims per group

_STATE = {}


def _build(loop_r=1):
    """Build the Bass program (shared by all 8 cores; inputs differ)."""
    from contextlib import ExitStack

    import concourse.tile as tile
    from concourse import bacc, mybir

    F32 = mybir.dt.float32
    BF16 = mybir.dt.bfloat16
    EXP = mybir.ActivationFunctionType.Exp

    nc = bacc.Bacc("TRN2", target_bir_lowering=False, debug=False,
                   num_devices=NCORES)

    qT_ext = nc.dram_tensor("qT", [D, S], BF16, kind="ExternalInput").ap()
    kT_ext = nc.dram_tensor("kT", [D, S], BF16, kind="ExternalInput").ap()
    vT_ext = nc.dram_tensor("vT", [D, S], BF16, kind="ExternalInput").ap()
    wqT_ext = nc.dram_tensor("wqT", [D, GH], BF16, kind="ExternalInput").ap()
    wkT_ext = nc.dram_tensor("wkT", [D, GH], BF16, kind="ExternalInput").ap()
    wvT_ext = nc.dram_tensor("wvT", [D, GH], BF16, kind="ExternalInput").ap()
    woT_ext = nc.dram_tensor("woT", [GH, D], BF16, kind="ExternalInput").ap()
    bq_ext = nc.dram_tensor("bq", [GH, 1], F32, kind="ExternalInput").ap()
    bk_ext = nc.dram_tensor("bk", [GH, 1], F32, kind="ExternalInput").ap()
    bv_ext = nc.dram_tensor("bv", [1, GH], BF16, kind="ExternalInput").ap()
    out_ext = nc.dram_tensor("out", [S, D], BF16,
                             kind="ExternalOutput").ap()

    with tile.TileContext(nc) as tc, ExitStack() as ctx:
        cst = ctx.enter_context(tc.tile_pool(name="cst", bufs=1))
        wp = ctx.enter_context(tc.tile_pool(name="wp", bufs=1))
        actp = ctx.enter_context(tc.tile_pool(name="actp", bufs=1))
        xsq = ctx.enter_context(tc.tile_pool(name="xsq", bufs=2))
        xsk = ctx.enter_context(tc.tile_pool(name="xsk", bufs=2))
        xsv = ctx.enter_context(tc.tile_pool(name="xsv", bufs=2))
        pp = ctx.enter_context(tc.tile_pool(name="pp", bufs=4))
        ob = ctx.enter_context(tc.tile_pool(name="ob", bufs=3))
        sm = ctx.enter_context(tc.tile_pool(name="sm", bufs=3))
        ps2 = ctx.enter_context(tc.tile_pool(name="ps2", bufs=2,
                                             space="PSUM"))
        ps1 = ctx.enter_context(tc.tile_pool(name="ps1", bufs=1,
                                             space="PSUM"))
        fac = ctx.enter_context(tc.tile_pool(name="fac", bufs=2,
                                             space="PSUM"))

        # ---- persistent tiles (addresses fixed across iterations) ----
        ones_f = cst.tile([128, 128], F32, tag="ones_f")
        nc.vector.memset(ones_f[:], 1.0)
        ones_b = cst.tile([128, 128], BF16, tag="ones_b")
        nc.vector.tensor_copy(ones_b[:], ones_f[:])

        bq_sb = cst.tile([128, 2], F32, tag="bq_sb")
        bk_sb = cst.tile([128, 2], F32, tag="bk_sb")
        bv_sb = cst.tile([1, GH], BF16, tag="bv_sb")
        bv128 = cst.tile([128, GH], BF16, tag="bv128")

        wq_sb = wp.tile([128, ND * GH], BF16, tag="wq_sb")
        wk_sb = wp.tile([128, ND * GH], BF16, tag="wk_sb")
        wv_sb = wp.tile([128, ND * GH], BF16, tag="wv_sb")
        wo_sb = wp.tile([128, 2 * D], BF16, tag="wo_sb")

        # generation g in {0,1}: khT[g][pr], vh[g][t]
        khT = [[actp.tile([128, S], BF16, tag=f"khT{g}_{i}",
                          name=f"khT{g}_{i}") for i in range(2)]
               for g in range(2)]
        vh = [[actp.tile([128, 4 * 128], BF16, tag=f"vh{g}_{t}",
                         name=f"vh{g}_{t}") for t in range(NT)]
              for g in range(2)]
        qhT = [[actp.tile([128, QC], BF16, tag=f"qhT{i}_{qc}",
                          name=f"qhT{i}_{qc}")
                for qc in range(NQC)] for i in range(2)]
        ctxT = [[actp.tile([128, QC], BF16, tag=f"ctxT{pr}_{qc}",
                           name=f"ctxT{pr}_{qc}")
                 for qc in range(NQC)] for pr in range(2)]

        qv = qT_ext.rearrange("(a p) s -> p a s", p=128)
        kv = kT_ext.rearrange("(a p) s -> p a s", p=128)
        vv = vT_ext.rearrange("(a p) s -> p a s", p=128)

        def load_weights():
            for dt_ in range(ND):
                sl = slice(dt_ * GH, (dt_ + 1) * GH)
                rows = slice(dt_ * 128, (dt_ + 1) * 128)
                nc.sync.dma_start(wk_sb[:, sl], wkT_ext[rows, :])
            for i in range(2):
                nc.sync.dma_start(bk_sb[:, i:i + 1],
                                  bk_ext[i * 128:(i + 1) * 128, :])
            for dt_ in range(ND):
                sl = slice(dt_ * GH, (dt_ + 1) * GH)
                rows = slice(dt_ * 128, (dt_ + 1) * 128)
                nc.sync.dma_start(wv_sb[:, sl], wvT_ext[rows, :])
            nc.sync.dma_start(bv_sb[:], bv_ext[:])
            for dt_ in range(ND):
                sl = slice(dt_ * GH, (dt_ + 1) * GH)
                rows = slice(dt_ * 128, (dt_ + 1) * 128)
                nc.sync.dma_start(wq_sb[:, sl], wqT_ext[rows, :])
            for i in range(2):
                nc.sync.dma_start(bq_sb[:, i:i + 1],
                                  bq_ext[i * 128:(i + 1) * 128, :])
            nc.sync.dma_start(wo_sb[:, 0:D], woT_ext[0:128, :])
            nc.sync.dma_start(wo_sb[:, D:2 * D], woT_ext[128:256, :])

        def stage(pool, view, c, tag):
            t = pool.tile([128, ND * QC], BF16, tag=tag, name=tag)
            nc.gpsimd.dma_start(
                t[:].rearrange("p (a s) -> p a s", a=ND),
                view[:, :, c * QC:(c + 1) * QC])
            return t

        def scores_mm(sl, gen, qc, pr, t):
            for hh in range(2):
                nc.tensor.matmul(
                    sl[:, hh * 512:hh * 512 + QC],
                    khT[gen][pr][hh * 64:(hh + 1) * 64,
                                 t * 128:(t + 1) * 128],
                    qhT[pr][qc][hh * 64:(hh + 1) * 64, :],
                    start=True, stop=True)

        # ---- filler op builders (each op emits one PE matmul; groups
        # accumulate in a rotating 1-bank [128,512] PSUM tile) ----
        def outproj_ops(pqc):
            cell = {}
            ops = []
            for st in range(4):
                for hf in range(2):
                    for pr2 in range(2):
                        def f(st=st, hf=hf, pr2=pr2):
                            if pr2 == 0:
                                cell[(st, hf)] = fac.tile(
                                    [128, 512], F32, tag="fa", name="fa")
                            acc = cell[(st, hf)]
                            nc.tensor.matmul(
                                acc[:],
                                ctxT[pr2][pqc][:, st * 128:(st + 1) * 128],
                                wo_sb[:, pr2 * D + hf * 512:
                                      pr2 * D + (hf + 1) * 512],
                                start=(pr2 == 0), stop=(pr2 == 1))
                            if pr2 == 1:
                                o_sb = ob.tile([128, 512], BF16,
                                               tag="o_sb", name="o_sb")
                                nc.vector.tensor_copy(o_sb[:], acc[:])
                                s_t = pqc * 4 + st
                                nc.sync.dma_start(
                                    out_ext[s_t * 128:(s_t + 1) * 128,
                                            hf * 512:(hf + 1) * 512],
                                    o_sb[:])
                        ops.append(f)
            return ops  # 16 ops

        def qkproj_ops(w_sb, b_sb, x_t, dst0, dst1):
            """Project one 512-col x chunk; dst_i gets half i + bias."""
            cell = {}
            ops = []
            for i in range(2):
                for dt_ in range(ND):
                    def f(i=i, dt_=dt_):
                        if dt_ == 0:
                            cell[i] = fac.tile([128, 512], F32,
                                               tag="fa", name="fa")
                        nc.tensor.matmul(
                            cell[i][:],
                            w_sb[:, dt_ * GH + i * 128:
                                 dt_ * GH + (i + 1) * 128],
                            x_t[:, dt_ * QC:(dt_ + 1) * QC],
                            start=(dt_ == 0), stop=(dt_ == ND - 1))
                        if dt_ == ND - 1:
                            nc.vector.tensor_scalar_add(
                                (dst0, dst1)[i], cell[i][:],
                                b_sb[:, i:i + 1])
                    ops.append(f)
            return ops  # 16 ops

        def qproj_ops(nqc, x_t):
            return qkproj_ops(wq_sb, bq_sb, x_t,
                              qhT[0][nqc][:], qhT[1][nqc][:])

        def kproj_ops(gw, c, x_t):
            return qkproj_ops(wk_sb, bk_sb, x_t,
                              khT[gw][0][:, c * QC:(c + 1) * QC],
                              khT[gw][1][:, c * QC:(c + 1) * QC])

        def vproj_ops(gw, g, x_t):
            cell = {}
            ops = []
            for st8 in range(4):
                for j in range(ND):
                    def f(st8=st8, j=j):
                        if j == 0:
                            cell[st8] = fac.tile([128, 512], F32,
                                                 tag="fa", name="fa")
                        acc = cell[st8]
                        nc.tensor.matmul(
                            acc[:, 0:256],
                            x_t[:, j * QC + st8 * 128:
                                j * QC + (st8 + 1) * 128],
                            wv_sb[:, j * GH:(j + 1) * GH],
                            start=(j == 0), stop=(j == ND - 1))
                        if j == ND - 1:
                            t_ = g * 4 + st8
                            dst4 = vh[gw][t_][:].rearrange(
                                "p (h c) -> p h c", h=4)
                            nc.vector.tensor_add(
                                dst4[:, :, 0:64],
                                acc[:, 0:256].rearrange(
                                    "p (h c) -> p h c", h=4),
                                bv128[:].rearrange(
                                    "p (h c) -> p h c", h=4))
                    ops.append(f)
            return ops  # 32 ops

        def normalize(pr, qc, ctx_ps):
            ctx_sb = sm.tile([128, 1024], F32, tag="ctx_sb", name="ctx_sb")
            # split halves: frees the PSUM bank for the next pass's PV(0)
            # in hh order (hh=0 writes [:, 0:512] first)
            nc.vector.tensor_copy(ctx_sb[:, 0:512], ctx_ps[:, 0:512])
            nc.vector.tensor_copy(ctx_sb[:, 512:1024], ctx_ps[:, 512:1024])
            den = sm.tile([128, 1024], F32, tag="den", name="den")
            nc.sync.dma_start(den[0:64, :], ctx_sb[64:128, :])
            rec = sm.tile([128, 1024], F32, tag="rec", name="rec")
            nc.vector.reciprocal_approx_fast(rec[0:64, :], den[0:64, :])
            nc.vector.tensor_mul(ctxT[pr][qc][0:64, :],
                                 ctx_sb[0:64, 0:QC], rec[0:64, 0:QC])
            bd = sm.tile([128, QC], BF16, tag="bd", name="bd")
            nc.vector.tensor_mul(bd[0:64, :],
                                 ctx_sb[0:64, QC:2 * QC],
                                 rec[0:64, QC:2 * QC])
            nc.sync.dma_start(ctxT[pr][qc][64:128, :], bd[0:64, :])

        pend = {"sl": None}
        cur = {}

        def attn_pass(gen, qc, pr, ops, nxt):
            """One head-pair pass; ops = flat filler list spread over the
            16 t-slots; nxt = (gen', qc', pr') whose scores(0) is issued
            in the last slot (or None)."""
            ctx_ps = ps1.tile([128, 1024], F32, tag="ctx", name="ctx")
            sls = {}
            if pend["sl"] is not None:
                sls[0] = pend.pop("sl")
                pend["sl"] = None
            else:
                sls[0] = ps2.tile([128, 1024], F32, tag="sl", name="sl")
                scores_mm(sls[0], gen, qc, pr, 0)
            nops = len(ops)
            for t in range(NT):
                if t + 1 < NT:
                    sls[t + 1] = ps2.tile([128, 1024], F32, tag="sl",
                                          name="sl")
                    scores_mm(sls[t + 1], gen, qc, pr, t + 1)
                elif nxt is not None:
                    s = ps2.tile([128, 1024], F32, tag="sl", name="sl")
                    scores_mm(s, *nxt, 0)
                    pend["sl"] = s
                sl = sls.pop(t)
                p = pp.tile([128, 1024], BF16, tag="p", name="p")
                nc.scalar.activation(p[:], sl[:], EXP)
                for f in ops[t * nops // NT:(t + 1) * nops // NT]:
                    f()
                for hh in range(2):
                    h = pr * 2 + hh
                    nc.tensor.matmul(
                        ctx_ps[:, hh * 512:hh * 512 + QC],
                        vh[gen][t][:, h * 128:(h + 1) * 128],
                        p[:, hh * 512:(hh + 1) * 512],
                        start=(t == 0), stop=(t == NT - 1))
            normalize(pr, qc, ctx_ps)

        def body(ph):
            gen = ph % 2
            ngen = 1 - gen
            for qc in range(NQC):
                xq_t = stage(xsq, qv, (qc + 1) % NQC, "xq")
                nk = stage(xsk, kv, (qc + 1) % NQC, "xk")
                nv = stage(xsv, vv, (qc + 1) % NQC, "xv")
                xk_t, xv_t = cur["xk"], cur["xv"]
                # out-proj of chunk qc-1 runs in the pr=1 pass so the
                # normalize tail (bd DMA into ctxT rows 64:128) of chunk
                # qc-1's pr=1 pass has a full pass of slack before its
                # first reader.
                ops0 = (qproj_ops((qc + 1) % NQC, xq_t)
                        + vproj_ops(ngen, qc, xv_t))
                ops1 = (kproj_ops(ngen, qc, xk_t)
                        + outproj_ops((qc - 1) % NQC))
                if qc == NQC - 1:
                    nxt1 = (ngen, 0, 0)
                else:
                    nxt1 = (gen, qc + 1, 0)
                attn_pass(gen, qc, 0, ops0, (gen, qc, 1))
                attn_pass(gen, qc, 1, ops1, nxt1)
                cur["xk"], cur["xv"] = nk, nv

        # ---- prologue ----
        for pr in range(2):
            nc.vector.memset(ctxT[pr][NQC - 1][:], 0.0)
        for g2 in range(2):
            for t in range(NT):
                nc.vector.tensor_copy(
                    vh[g2][t][:].rearrange(
                        "p (h c) -> p h c", h=4)[:, :, 64:128],
                    ones_b[:, 0:64].unsqueeze(1).broadcast_to((128, 4, 64)))
        load_weights()
        # replicate bv across all 128 partitions (ones-column matmul)
        bvp = fac.tile([128, 512], F32, tag="fa", name="fa")
        nc.tensor.matmul(bvp[:, 0:GH], ones_b[0:1, 0:128], bv_sb[:],
                         start=True, stop=True)
        nc.vector.tensor_copy(bv128[:], bvp[:, 0:GH])
        # build generation-0 khT/vh
        for c in range(NQC):
            xk_t = stage(xsk, kv, c, "xk")
            xv_t = stage(xsv, vv, c, "xv")
            for f in kproj_ops(0, c, xk_t):
                f()
            for f in vproj_ops(0, c, xv_t):
                f()
        cur["xk"] = stage(xsk, kv, 0, "xk")
        cur["xv"] = stage(xsv, vv, 0, "xv")
        x0 = stage(xsq, qv, 0, "xq")
        for f in qproj_ops(0, x0):
            f()
        sl0 = ps2.tile([128, 1024], F32, tag="sl", name="sl")
        scores_mm(sl0, 0, 0, 0, 0)
        pend["sl"] = sl0

        if loop_r > 1:
            assert loop_r % 2 == 0, "loop_r must be even (gen alternation)"
            u = 4 if loop_r % 4 == 0 else 2
            with tc.For_i(0, loop_r // u, 1):
                for ph in range(u):
                    body(ph)
        else:
            body(0)

        # Epilogue: project the final iteration's last chunk.
        for st in range(4):
            for hf in range(2):
                acc = fac.tile([128, 512], F32, tag="fa", name="fa")
                for pr2 in range(2):
                    nc.tensor.matmul(
                        acc[:],
                        ctxT[pr2][NQC - 1][:, st * 128:(st + 1) * 128],
                        wo_sb[:, pr2 * D + hf * 512:pr2 * D + (hf + 1) * 512],
                        start=(pr2 == 0), stop=(pr2 == 1))
                o_sb = ob.tile([128, 512], BF16, tag="o_sb", name="o_sb")
                nc.vector.tensor_copy(o_sb[:], acc[:])
                s_t = (NQC - 1) * 4 + st
                nc.sync.dma_start(
                    out_ext[s_t * 128:(s_t + 1) * 128,
                            hf * 512:(hf + 1) * 512], o_sb[:])

    nc.compile()
    return nc


class _Runner:
    """SPMD runner on 8 cores via the axon PJRT path (no re-trace)."""

    def __init__(self, nc, n_cores):
        import jax
        from jax.sharding import Mesh, PartitionSpec
        from jax.experimental.shard_map import shard_map
        import concourse.mybir as mybir
        from concourse import bass2jax

        bass2jax.install_neuronx_cc_hook()
        self._jax = jax
        pname = nc.partition_id_tensor.name if nc.partition_id_tensor else None
        in_names, out_names, out_avals, zero_outs = [], [], [], []
        for alloc in nc.m.functions[0].allocations:
            if not isinstance(alloc, mybir.MemoryLocationSet):
                continue
            name = alloc.memorylocations[0].name
            if alloc.kind == "ExternalInput":
                if name != pname:
                    in_names.append(name)
            elif alloc.kind == "ExternalOutput":
                shape = tuple(alloc.tensor_shape)
                dtype = mybir.dt.np(alloc.dtype)
                out_names.append(name)
                out_avals.append(jax.core.ShapedArray(shape, dtype))
                zero_outs.append(np.zeros(shape, dtype))
        self.in_names, self.out_names = in_names, out_names
        self.out_avals, self.zero_outs = out_avals, zero_outs
        self.n_cores = n_cores
        all_in = list(in_names) + list(out_names) + ([pname] if pname else [])

        def _body(*args):
            operands = list(args)
            if pname is not None:
                operands.append(bass2jax.partition_id_tensor())
            return tuple(bass2jax._bass_exec_p.bind(
                *operands, out_avals=tuple(out_avals), in_names=tuple(all_in),
                out_names=tuple(out_names), lowering_input_output_aliases=(),
                sim_require_finite=True, sim_require_nnan=True, nc=nc))

        devices = jax.devices()[:n_cores]
        assert len(devices) >= 1
        self.mesh = Mesh(np.asarray(devices), ("core",))
        spec = PartitionSpec("core")
        n_args = len(in_names) + len(out_names)
        self.fn = jax.jit(
            shard_map(_body, mesh=self.mesh, in_specs=(spec,) * n_args,
                      out_specs=(spec,) * len(out_names), check_rep=False),
            keep_unused=True)
        self.sharding = jax.sharding.NamedSharding(self.mesh, spec)

    def put_inputs(self, in_maps):
        jax = self._jax
        args = []
        for name in self.in_names:
            cat = np.concatenate([np.ascontiguousarray(m[name])
                                  for m in in_maps], axis=0)
            args.append(jax.device_put(cat, self.sharding))
        for z in self.zero_outs:
            cat = np.zeros((self.n_cores * z.shape[0], *z.shape[1:]), z.dtype)
            args.append(jax.device_put(cat, self.sharding))
        return args

    def run(self, args):
        outs = self.fn(*args)
        self._jax.block_until_ready(outs)
        return outs

    def results(self, outs):
        res = []
        for c in range(self.n_cores):
            d = {}
            for i, name in enumerate(self.out_names):
                d[name] = np.asarray(outs[i]).reshape(
                    self.n_cores, *self.out_avals[i].shape)[c]
            res.append(d)
        return res


def _make_in_maps(q, k, v, wq, bq, wk, bk, wv, bv, wo):
    """Host-side sharding/layout prep. Core c = b*4 + g."""
    import ml_dtypes
    BF = ml_dtypes.bfloat16
    scale = 1.0 / math.sqrt(DK)
    wq_s = (wq * scale).astype(np.float32)
    bq_s = (bq * scale).astype(np.float32)
    xT = {}
    for b in range(B):
        xT["q", b] = np.ascontiguousarray(q[b].T).astype(BF)
        xT["k", b] = np.ascontiguousarray(k[b].T).astype(BF)
        xT["v", b] = np.ascontiguousarray(v[b].T).astype(BF)
    in_maps = []
    for c in range(NCORES):
        b, g = divmod(c, HPC)
        hd = slice(g * GH, (g + 1) * GH)
        in_maps.append({
            "qT": xT["q", b],
            "kT": xT["k", b],
            "vT": xT["v", b],
            "wqT": np.ascontiguousarray(wq_s[hd, :].T).astype(BF),
            "wkT": np.ascontiguousarray(wk[hd, :].T).astype(BF),
            "wvT": np.ascontiguousarray(wv[hd, :].T).astype(BF),
            "woT": np.ascontiguousarray(wo[:, hd].T).astype(BF),
            "bq": np.ascontiguousarray(bq_s[hd].reshape(GH, 1)),
            "bk": np.ascontiguousarray(bk[hd].reshape(GH, 1)),
            "bv": np.ascontiguousarray(bv[hd].reshape(1, GH)).astype(BF),
        })
    return in_maps


def _numpy_reference(q, k, v, mask, wq, bq, wk, bk, wv, bv, wo, bo):
    """Exact fp32 fallback (only used if mask has zeros)."""
    qh = (q @ wq.T + bq).reshape(B, S, H, DK).transpose(0, 2, 1, 3)
    kh = (k @ wk.T + bk).reshape(B, S, H, DK).transpose(0, 2, 1, 3)
    vh = (v @ wv.T + bv).reshape(B, S, H, DK).transpose(0, 2, 1, 3)
    out = np.zeros((B, S, D), np.float32)
    for b in range(B):
        for h in range(H):
            sc = (qh[b, h] @ kh[b, h].T) / math.sqrt(DK)
            sc = np.where(mask[0, 0] == 0, np.float32(-1e9), sc)
            sc = sc - sc.max(axis=-1, keepdims=True)
            e = np.exp(sc)
            p = e / e.sum(axis=-1, keepdims=True)
            out[b, :, h * DK:(h + 1) * DK] = p @ vh[b, h]
    return out.reshape(B * S, D) @ wo.T + bo


def get_runner(loop_r=1):
    key = ("runner", loop_r)
    if key not in _STATE:
        nc = _build(loop_r=loop_r)
        _STATE[key] = _Runner(nc, NCORES)
    return _STATE[key]


def kernel(q, k, v, mask, wq, bq, wk, bk, wv, bv, wo, bo):
    q = np.asarray(q, np.float32)
    k = np.asarray(k, np.float32)
    v = np.asarray(v, np.float32)
    mask = np.asarray(mask)
    wq = np.asarray(wq, np.float32); bq = np.asarray(bq, np.float32)
    wk = np.asarray(wk, np.float32); bk = np.asarray(bk, np.float32)
    wv = np.asarray(wv, np.float32); bv = np.asarray(bv, np.float32)
    wo = np.asarray(wo, np.float32); bo = np.asarray(bo, np.float32)

    if np.any(mask == 0):
        out = _numpy_reference(q, k, v, mask, wq, bq, wk, bk, wv, bv, wo, bo)
        return out.reshape(B, S, D).astype(np.float32)

    r = get_runner()
    in_maps = _make_in_maps(q, k, v, wq, bq, wk, bk, wv, bv, wo)
    outs = r.run(r.put_inputs(in_maps))
    res = r.results(outs)
    full = np.zeros((B, S, D), np.float32)
    for c in range(NCORES):
        b = c // HPC
        full[b] += res[c]["out"]
    full += bo[None, None, :]
    return full
